# revision 1
# baseline (speedup 1.0000x reference)
"""Causal multi-head attention kernel for Trainium2, 8 NeuronCores.

Problem: x[4,2048,1024] fp32, Wq/Wk/Wv/Wo[1024,1024], bo[1024].
  y = softmax(causal(Q K^T)/sqrt(64)) V @ Wo + bo, H=16 heads of D=64.

Sharding (per hint): data-parallel over batch (4) x tensor-parallel over
heads (2 groups of 8). Core c handles batch c//2, heads (c%2)*8..+8:
Wq/Wk/Wv column-sharded [1024,512], Wo row-sharded [512,1024], pairwise
AllReduce after out_proj, chunked per q-tile so communication overlaps
the next tile's compute.

Per-core layout strategy:
  - x^T is provided by the host (numpy transpose), so every projection
    matmul (contraction over E) reads it directly; no on-device
    transposes at all.
  - All matmul operands use float32r: full-rate fp32 PE mode (~13-bit
    operand mantissa, fp32 accumulate; measured ~5e-5 matmul rel err).
    Inputs are declared float32r so DMA'd data feeds matmuls directly.
  - Q^T, K^T stored [128, S] per head-pair (rows 0-63 = head 2hp,
    64-127 = head 2hp+1).
  - Scores computed transposed: S^T[k, q] = K Q^T per (128-k-block x
    512-q-tile); the two heads run as row-packed concurrent K=64
    matmuls into adjacent PSUM banks.
  - exp on ACT reads both banks [128,1024] in one instruction and
    writes the P~ slab; causal masking multiplies the 4 diagonal-band
    blocks by slices of one precomputed step-pattern ("mega-mask").
  - AV accumulated transposed: O^T[d, q] += V[kb].T @ P~[kb], heads
    col-packed into one PSUM bank; softmax denominators via ones-column
    matmuls (normalization deferred to after AV).
  - Normalization: reciprocal of sums, partition-broadcast via K=1
    ones matmul, multiply into A^T (which is exactly the out-proj lhsT
    layout).
  - Out-proj accumulates 4 head-pair chunks + a K=1 bias matmul
    (bo/2 per TP rank), then DMA to DRAM, pairwise chunked AllReduce.

Requires bacc.Bacc (not bass.Bass): its generate_event_semaphores pass
splits multi-waits (TRN2 allows one sync wait per instruction).
"""

import numpy as np

import concourse.bass as bass
from concourse import bacc
import concourse.mybir as mybir
import concourse.tile as tile
from concourse.bass_utils import run_bass_kernel_spmd

B, S, E, H, D = 4, 2048, 1024, 16, 64
ESH = 512           # per-core E shard (8 heads x 64)
HP = 4              # head pairs per core
NJ, QTW = 4, 512    # q tiles
NKB, KBW = 16, 128  # k blocks

fp32 = mybir.dt.float32
f32r = mybir.dt.float32r
DT = f32r           # matmul operand dtype (f32r = fast fp32 PE mode)
AF = mybir.ActivationFunctionType


def _body(tc, io):
    nc = tc.nc

    const_cm = tc.tile_pool(name="const", bufs=1)
    const = const_cm.__enter__()
    ones_st = const.tile([33, 64], fp32, tag="ones_st")
    nc.vector.memset(ones_st, 1.0)
    ones = const.tile([33, 64], DT, tag="ones")
    nc.vector.tensor_copy(ones, ones_st)
    oner_st = const.tile([1, 128], fp32, tag="oner_st")
    nc.vector.memset(oner_st, 1.0)
    onecol_st = const.tile([128, 8], fp32, tag="onecol_st")
    nc.vector.memset(onecol_st, 1.0)
    ones_row = const.tile([1, 128], DT, tag="ones_row")
    nc.vector.tensor_copy(ones_row, oner_st)
    # mega-mask M[kp,u] = (u >= kp+384); diag pattern r slice = [384-128r:+512]
    masks = const.tile([128, 896], DT, tag="masks")
    nc.sync.dma_start(out=masks, in_=io["masks"].ap())
    bo_dt = const.tile([1, E], DT, tag="bo_dt")
    nc.sync.dma_start(out=bo_dt, in_=io["bo"].ap()[None, :])
    # Wo stays resident through phase 3; Wq/Wk/Wv live in a phase-1 pool
    wo_sb = const.tile([128, 4, E], DT, tag="wo")
    nc.sync.dma_start(out=wo_sb, in_=io["wo"].ap().rearrange("(c p) e -> p c e", p=128))

    kv_cm = tc.tile_pool(name="kv", bufs=1)
    kv = kv_cm.__enter__()
    qt_sb = [kv.tile([128, S], DT, tag=f"qt{hp}", name=f"qt{hp}") for hp in range(HP)]
    kt_sb = [kv.tile([128, S], DT, tag=f"kt{hp}", name=f"kt{hp}") for hp in range(HP)]
    v_sb = [kv.tile([128, 8, 65], DT, tag=f"v{kb}", name=f"v{kb}") for kb in range(NKB)]

    # ---------------- phase 1: projections ----------------
    wpool_cm = tc.tile_pool(name="wpool", bufs=1)
    wpool = wpool_cm.__enter__()
    wq_sb = wpool.tile([128, 8, ESH], DT, tag="wq")
    wk_sb = wpool.tile([128, 8, ESH], DT, tag="wk")
    wv_sb = wpool.tile([128, 8, ESH], DT, tag="wv")
    for t, nm in ((wq_sb, "wq"), (wk_sb, "wk"), (wv_sb, "wv")):
        nc.sync.dma_start(out=t, in_=io[nm].ap().rearrange("(e p) c -> p e c", p=128))

    xpool_cm = tc.tile_pool(name="xpool", bufs=2)
    xpool = xpool_cm.__enter__()
    ps1_cm = tc.tile_pool(name="ps1", bufs=1, space="PSUM")
    ps1 = ps1_cm.__enter__()

    for st_i in range(4):
        ssl = slice(st_i * 512, (st_i + 1) * 512)
        # x^T page for this s-tile: [128, 8 E-chunks, 512]
        xt = xpool.tile([128, 8, 512], DT, tag="xt")
        nc.sync.dma_start(
            out=xt,
            in_=io["xt"].ap().rearrange("(e p) s -> p e s", p=128)[:, :, ssl])
        for hp in range(HP):
            psq = ps1.tile([128, 512], fp32, tag="pj", bufs=3)
            for e in range(8):
                nc.tensor.matmul(psq, wq_sb[:, e, hp * 128:(hp + 1) * 128],
                                 xt[:, e, :], start=(e == 0), stop=(e == 7))
            nc.vector.tensor_copy(qt_sb[hp][:, ssl], psq)
            psk = ps1.tile([128, 512], fp32, tag="pj", bufs=3)
            for e in range(8):
                nc.tensor.matmul(psk, wk_sb[:, e, hp * 128:(hp + 1) * 128],
                                 xt[:, e, :], start=(e == 0), stop=(e == 7))
            nc.vector.tensor_copy(kt_sb[hp][:, ssl], psk)
        for sb in range(4):
            psv = ps1.tile([128, 512], fp32, tag="pj", bufs=3)
            for e in range(8):
                nc.tensor.matmul(psv, xt[:, e, sb * 128:(sb + 1) * 128],
                                 wv_sb[:, e, :], start=(e == 0), stop=(e == 7))
            kb = st_i * 4 + sb
            nc.vector.tensor_copy(
                v_sb[kb][:, :, 0:64],
                psv.rearrange("p (h d) -> p h d", h=8))
            nc.vector.tensor_copy(
                v_sb[kb][:, :, 64:65],
                onecol_st.rearrange("p h -> p h 1" if False else "p (h o) -> p h o", o=1))

    ps1_cm.__exit__(None, None, None)
    xpool_cm.__exit__(None, None, None)
    wpool_cm.__exit__(None, None, None)

    # -------- phase 2+3: attention, out-proj, chunked allreduce --------
    apool_cm = tc.tile_pool(name="apool", bufs=1)
    apool = apool_cm.__enter__()
    at_sb = [apool.tile([128, S], DT, tag=f"at{hp}", name=f"at{hp}") for hp in range(HP)]

    slabs_cm = tc.tile_pool(name="slabs", bufs=3)
    slabs = slabs_cm.__enter__()
    rpool_cm = tc.tile_pool(name="rpool", bufs=2)
    rpool = rpool_cm.__enter__()
    ypool_cm = tc.tile_pool(name="ypool", bufs=3)
    ypool = ypool_cm.__enter__()
    ps2_cm = tc.tile_pool(name="ps2", bufs=1, space="PSUM")
    ps2 = ps2_cm.__enter__()
    dram_cm = tc.tile_pool(name="dram", bufs=1, space="DRAM")
    dram = dram_cm.__enter__()
    ypart = dram.tile([S, E], fp32)
    ysum = dram.tile([S, E], fp32)

    for j in range(NJ):
        jsl = slice(j * QTW, (j + 1) * QTW)
        for hp in range(HP):
            # f32r matmuls reject col-tiled outputs (base partition 64), so
            # the two heads accumulate in separate base-0 PSUM tiles; head
            # B's normalized rows reach at_sb partitions 64-127 via a
            # partition-shifting SBUF->SBUF DMA afterwards.
            avA = ps2.tile([65, 512], fp32, tag="avA", bufs=1)
            avB = ps2.tile([65, 512], fp32, tag="avB", bufs=1)
            kmax = 4 * j + 4
            for kb in range(kmax):
                ksl = slice(kb * KBW, (kb + 1) * KBW)
                sc = ps2.tile([128, 1024], fp32, tag="sc", bufs=2)
                nc.tensor.matmul(sc[:, 0:512], kt_sb[hp][0:64, ksl],
                                 qt_sb[hp][0:64, jsl], start=True, stop=True)
                nc.tensor.matmul(sc[:, 512:1024], kt_sb[hp][64:128, ksl],
                                 qt_sb[hp][64:128, jsl], start=True, stop=True)
                slab = slabs.tile([128, 1024], DT, tag="slab")
                nc.scalar.activation(slab, sc, AF.Exp, bias=0.0, scale=0.125)
                r = kb - 4 * j
                if r >= 0:
                    msl = slice(384 - 128 * r, 384 - 128 * r + 512)
                    nc.vector.tensor_mul(slab[:, 0:512], slab[:, 0:512],
                                         masks[:, msl])
                    nc.vector.tensor_mul(slab[:, 512:1024], slab[:, 512:1024],
                                         masks[:, msl])
                first, last = kb == 0, kb == kmax - 1
                nc.tensor.matmul(avA, v_sb[kb][:, 2 * hp, :],
                                 slab[:, 0:512], start=first, stop=last)
                nc.tensor.matmul(avB, v_sb[kb][:, 2 * hp + 1, :],
                                 slab[:, 512:1024], start=first, stop=last)
            recipA = rpool.tile([1, 512], DT, tag="recipA")
            recipB = rpool.tile([1, 512], DT, tag="recipB")
            with nc.allow_low_precision(reason="f32r recip, ~1e-6 rel err"):
                nc.vector.reciprocal(recipA, avA[64:65, :])
                nc.vector.reciprocal(recipB, avB[64:65, :])
            bcA = ps2.tile([64, 512], fp32, tag="bcA", bufs=1)
            bcB = ps2.tile([64, 512], fp32, tag="bcB", bufs=1)
            nc.tensor.matmul(bcA, ones[0:1, :], recipA, start=True, stop=True)
            nc.tensor.matmul(bcB, ones[0:1, :], recipB, start=True, stop=True)
            # DVE reads at most one PSUM operand: stage bc in SBUF
            bcA_sb = rpool.tile([64, 512], DT, tag="bcA_sb")
            nc.vector.tensor_copy(bcA_sb, bcA)
            bcB_sb = rpool.tile([64, 512], DT, tag="bcB_sb")
            nc.vector.tensor_copy(bcB_sb, bcB)
            nc.vector.tensor_mul(at_sb[hp][0:64, jsl], avA[0:64, :], bcA_sb)
            atB = rpool.tile([64, 512], DT, tag="atB")
            nc.vector.tensor_mul(atB, avB[0:64, :], bcB_sb)
            nc.sync.dma_start(out=at_sb[hp][64:128, jsl], in_=atB)
        # out-proj for q-tile j (+ bias via K=1 accumulate; bo pre-halved)
        for qs in range(4):
            q0 = j * QTW + qs * 128
            yp = ps2.tile([128, 1024], fp32, tag="sc", bufs=2)
            for half in range(2):
                hsl = slice(half * 512, (half + 1) * 512)
                for hp in range(HP):
                    nc.tensor.matmul(
                        yp[:, hsl], at_sb[hp][:, q0:q0 + 128],
                        wo_sb[:, hp, hsl], start=(hp == 0), stop=False)
                nc.tensor.matmul(yp[:, hsl], ones_row, bo_dt[0:1, hsl],
                                 start=False, stop=True)
            ysb = ypool.tile([128, E], fp32, tag="ysb")
            nc.vector.tensor_copy(ysb, yp)
            nc.sync.dma_start(out=ypart[q0:q0 + 128, :], in_=ysb)
        nc.gpsimd.collective_compute(
            "AllReduce", mybir.AluOpType.add,
            replica_groups=[[0, 1], [2, 3], [4, 5], [6, 7]],
            ins=[ypart[jsl, :].opt()], outs=[ysum[jsl, :].opt()],
        )
        nc.sync.dma_start(out=io["y"].ap()[jsl, :], in_=ysum[jsl, :])

    for cm in (dram_cm, ps2_cm, ypool_cm, rpool_cm, slabs_cm, apool_cm,
               kv_cm, const_cm):
        cm.__exit__(None, None, None)


def build():
    nc = bacc.Bacc("TRN2", target_bir_lowering=False, debug=False,
                   num_devices=8)
    io = {
        "xt": nc.dram_tensor("xt", [E, S], f32r, kind="ExternalInput"),
        "wq": nc.dram_tensor("wq", [E, ESH], f32r, kind="ExternalInput"),
        "wk": nc.dram_tensor("wk", [E, ESH], f32r, kind="ExternalInput"),
        "wv": nc.dram_tensor("wv", [E, ESH], f32r, kind="ExternalInput"),
        "wo": nc.dram_tensor("wo", [ESH, E], f32r, kind="ExternalInput"),
        "bo": nc.dram_tensor("bo", [E], f32r, kind="ExternalInput"),
        "masks": nc.dram_tensor("masks", [128, 896], f32r, kind="ExternalInput"),
        "y": nc.dram_tensor("y", [S, E], fp32, kind="ExternalOutput"),
    }
    with tile.TileContext(nc) as tc:
        _body(tc, io)
    nc.finalize()
    return nc


def make_in_maps(x, Wq, Wk, Wv, Wo, bo):
    """Shard full inputs into the 8 per-core input maps."""
    x = np.asarray(x, dtype=np.float32)
    Wq, Wk, Wv, Wo = (np.asarray(w, dtype=np.float32) for w in (Wq, Wk, Wv, Wo))
    bo = np.asarray(bo, dtype=np.float32)
    kp = np.arange(128)[:, None]
    u = np.arange(896)[None, :]
    masks = (u >= kp + 384).astype(np.float32)
    in_maps = []
    for c in range(8):
        b, g = c // 2, c % 2
        csl = slice(g * ESH, (g + 1) * ESH)
        in_maps.append({
            "xt": np.ascontiguousarray(x[b].T),
            "wq": np.ascontiguousarray(Wq[:, csl]),
            "wk": np.ascontiguousarray(Wk[:, csl]),
            "wv": np.ascontiguousarray(Wv[:, csl]),
            "wo": np.ascontiguousarray(Wo[csl, :]),
            "bo": np.ascontiguousarray(bo * 0.5),
            "masks": masks,
        })
    return in_maps


def kernel(x, Wq, Wk, Wv, Wo, bo):
    nc = build()
    in_maps = make_in_maps(x, Wq, Wk, Wv, Wo, bo)
    res = run_bass_kernel_spmd(nc, in_maps, core_ids=list(range(8)))
    y = np.empty((B, S, E), dtype=np.float32)
    for b in range(B):
        y[b] = res.results[2 * b]["y"]
    return y



# revision 2
# speedup vs baseline: 3.4161x; 3.4161x over previous
"""Causal multi-head attention kernel for Trainium2, 8 NeuronCores — v2.

Problem: x[4,2048,1024] fp32, Wq/Wk/Wv/Wo[1024,1024], bo[1024].
  y = softmax(causal(Q K^T)/sqrt(64)) V @ Wo + bo, H=16 heads of D=64.

Sharding: data-parallel over batch (4) x tensor-parallel over heads
(2 groups of 8). Core c handles batch c//2, heads (c%2)*8..+8.

v2 changes vs v1 (which AllReduced full y per q-tile chunk):
  - All I/O shipped as bf16 (halves per-execute host<->device traffic,
    which dominates the wall clock under the axon relay).
  - All matmuls run in bf16 (PE full rate, fp32 PSUM accumulation).
  - Out-proj: instead of computing full y partial sums + AllReduce
    (2N wire), each TP pair AllGathers the normalized attention
    outputs A^T per q-tile (N/2 wire) and each rank computes only its
    512 output columns: y[:, g*512:+512] = A_full @ Wo[:, g-half].
    The gathered A^T is read back uniformly on both ranks (same
    program), so out-proj consumes only the gathered copy.
  - Each core returns a [2048, 512] bf16 y column-shard (quarter the
    v1 output bytes); the host assembles and casts to fp32.

Per-core layout (unchanged core ideas from v1):
  - x^T shipped pre-transposed; projections contract over E directly.
  - Q^T/K^T stored [128, S] per local head-pair (rows 0-63 = head 2hp,
    64-127 = head 2hp+1); scores computed transposed as K Q^T with the
    two heads running as row-packed concurrent K=64 matmuls.
  - exp on ACT reads both PSUM banks [128,1024] in one instruction;
    causal masking multiplies the 4 diagonal-band blocks by slices of
    one precomputed step-pattern ("mega-mask").
  - AV accumulated transposed with a ones-column per V tile producing
    softmax denominators in row 64; normalization deferred to after AV
    via reciprocal + K=1 ones-matmul partition broadcast.
"""

import numpy as np
import ml_dtypes

import concourse.bass as bass
from concourse import bacc
import concourse.mybir as mybir
import concourse.tile as tile
from concourse.bass_utils import run_bass_kernel_spmd

B, S, E, H, D = 4, 2048, 1024, 16, 64
ESH = 512           # per-core E shard (8 heads x 64)
HP = 4              # head pairs per core
NJ, QTW = 4, 512    # q tiles
NKB, KBW = 16, 128  # k blocks

fp32 = mybir.dt.float32
bf16 = mybir.dt.bfloat16
DT = bf16
NPDT = ml_dtypes.bfloat16
AF = mybir.ActivationFunctionType
PAIRS = [[0, 1], [2, 3], [4, 5], [6, 7]]


def _body(tc, io):
    nc = tc.nc

    const_cm = tc.tile_pool(name="const", bufs=1)
    const = const_cm.__enter__()
    ones = const.tile([1, 64], DT, tag="ones")
    nc.vector.memset(ones, 1.0)
    ones_row = const.tile([1, 128], DT, tag="ones_row")
    nc.vector.memset(ones_row, 1.0)
    # mega-mask M[kp,u] = (u >= kp+384); diag pattern r slice = [384-128r:+512]
    masks = const.tile([128, 896], DT, tag="masks")
    nc.sync.dma_start(out=masks, in_=io["masks"].ap())
    bo_dt = const.tile([1, ESH], DT, tag="bo_dt")
    nc.sync.dma_start(out=bo_dt, in_=io["bo"].ap()[None, :])
    # Wo: [1024 rows = all 8 global head-pair chunks, 512 cols = g-half]
    wo_sb = const.tile([128, 8, ESH], DT, tag="wo")
    nc.sync.dma_start(out=wo_sb, in_=io["wo"].ap().rearrange("(c p) e -> p c e", p=128))

    kv_cm = tc.tile_pool(name="kv", bufs=1)
    kv = kv_cm.__enter__()
    qt_sb = [kv.tile([128, S], DT, tag=f"qt{hp}", name=f"qt{hp}") for hp in range(HP)]
    kt_sb = [kv.tile([128, S], DT, tag=f"kt{hp}", name=f"kt{hp}") for hp in range(HP)]
    v_sb = [kv.tile([128, 8, 65], DT, tag=f"v{kb}", name=f"v{kb}") for kb in range(NKB)]

    # ---------------- phase 1: projections ----------------
    wpool_cm = tc.tile_pool(name="wpool", bufs=1)
    wpool = wpool_cm.__enter__()
    wq_sb = wpool.tile([128, 8, ESH], DT, tag="wq")
    wk_sb = wpool.tile([128, 8, ESH], DT, tag="wk")
    wv_sb = wpool.tile([128, 8, ESH], DT, tag="wv")
    for t, nm in ((wq_sb, "wq"), (wk_sb, "wk"), (wv_sb, "wv")):
        nc.sync.dma_start(out=t, in_=io[nm].ap().rearrange("(e p) c -> p e c", p=128))

    xpool_cm = tc.tile_pool(name="xpool", bufs=2)
    xpool = xpool_cm.__enter__()
    ps1_cm = tc.tile_pool(name="ps1", bufs=1, space="PSUM")
    ps1 = ps1_cm.__enter__()

    for st_i in range(4):
        ssl = slice(st_i * 512, (st_i + 1) * 512)
        # x^T page for this s-tile: [128, 8 E-chunks, 512]
        xt = xpool.tile([128, 8, 512], DT, tag="xt")
        nc.sync.dma_start(
            out=xt,
            in_=io["xt"].ap().rearrange("(e p) s -> p e s", p=128)[:, :, ssl])
        for hp in range(HP):
            psq = ps1.tile([128, 512], fp32, tag="pj", bufs=3)
            for e in range(8):
                nc.tensor.matmul(psq, wq_sb[:, e, hp * 128:(hp + 1) * 128],
                                 xt[:, e, :], start=(e == 0), stop=(e == 7))
            nc.vector.tensor_copy(qt_sb[hp][:, ssl], psq)
            psk = ps1.tile([128, 512], fp32, tag="pj", bufs=3)
            for e in range(8):
                nc.tensor.matmul(psk, wk_sb[:, e, hp * 128:(hp + 1) * 128],
                                 xt[:, e, :], start=(e == 0), stop=(e == 7))
            nc.vector.tensor_copy(kt_sb[hp][:, ssl], psk)
        for sb in range(4):
            psv = ps1.tile([128, 512], fp32, tag="pj", bufs=3)
            for e in range(8):
                nc.tensor.matmul(psv, xt[:, e, sb * 128:(sb + 1) * 128],
                                 wv_sb[:, e, :], start=(e == 0), stop=(e == 7))
            kb = st_i * 4 + sb
            nc.vector.tensor_copy(
                v_sb[kb][:, :, 0:64],
                psv.rearrange("p (h d) -> p h d", h=8))
            nc.vector.memset(v_sb[kb][:, :, 64:65], 1.0)

    ps1_cm.__exit__(None, None, None)
    xpool_cm.__exit__(None, None, None)
    wpool_cm.__exit__(None, None, None)

    # -------- phase 2+3: attention, A^T AllGather, half out-proj --------
    slabs_cm = tc.tile_pool(name="slabs", bufs=3)
    slabs = slabs_cm.__enter__()
    rpool_cm = tc.tile_pool(name="rpool", bufs=2)
    rpool = rpool_cm.__enter__()
    afull_cm = tc.tile_pool(name="afull", bufs=2)
    afull = afull_cm.__enter__()
    ypool_cm = tc.tile_pool(name="ypool", bufs=3)
    ypool = ypool_cm.__enter__()
    ps2_cm = tc.tile_pool(name="ps2", bufs=1, space="PSUM")
    ps2 = ps2_cm.__enter__()
    dram_cm = tc.tile_pool(name="dram", bufs=1, space="DRAM")
    dram = dram_cm.__enter__()
    apart = dram.tile([NJ, ESH, QTW], DT)    # local A^T per q-tile
    agath = dram.tile([NJ, 2 * ESH, QTW], DT)  # pair-gathered A^T

    for j in range(NJ):
        jsl = slice(j * QTW, (j + 1) * QTW)
        for hp in range(HP):
            # two heads accumulate in separate base-0 PSUM tiles; row 64
            # collects softmax denominators via the V ones-column.
            avA = ps2.tile([65, 512], fp32, tag="avA", bufs=1)
            avB = ps2.tile([65, 512], fp32, tag="avB", bufs=1)
            kmax = 4 * j + 4
            for kb in range(kmax):
                ksl = slice(kb * KBW, (kb + 1) * KBW)
                sc = ps2.tile([128, 1024], fp32, tag="sc", bufs=2)
                nc.tensor.matmul(sc[:, 0:512], kt_sb[hp][0:64, ksl],
                                 qt_sb[hp][0:64, jsl], start=True, stop=True)
                nc.tensor.matmul(sc[:, 512:1024], kt_sb[hp][64:128, ksl],
                                 qt_sb[hp][64:128, jsl], start=True, stop=True)
                slab = slabs.tile([128, 1024], DT, tag="slab")
                nc.scalar.activation(slab, sc, AF.Exp, bias=0.0, scale=0.125)
                r = kb - 4 * j
                if r >= 0:
                    msl = slice(384 - 128 * r, 384 - 128 * r + 512)
                    nc.vector.tensor_mul(slab[:, 0:512], slab[:, 0:512],
                                         masks[:, msl])
                    nc.vector.tensor_mul(slab[:, 512:1024], slab[:, 512:1024],
                                         masks[:, msl])
                first, last = kb == 0, kb == kmax - 1
                nc.tensor.matmul(avA, v_sb[kb][:, 2 * hp, :],
                                 slab[:, 0:512], start=first, stop=last)
                nc.tensor.matmul(avB, v_sb[kb][:, 2 * hp + 1, :],
                                 slab[:, 512:1024], start=first, stop=last)
            recipA = rpool.tile([1, 512], DT, tag="recipA")
            recipB = rpool.tile([1, 512], DT, tag="recipB")
            with nc.allow_low_precision(reason="bf16 softmax denom recip"):
                nc.vector.reciprocal(recipA, avA[64:65, :])
                nc.vector.reciprocal(recipB, avB[64:65, :])
            bcA = ps2.tile([64, 512], fp32, tag="bc", bufs=1)
            nc.tensor.matmul(bcA, ones[0:1, :], recipA, start=True, stop=True)
            # DVE reads at most one PSUM operand: stage bc in SBUF
            bcA_sb = rpool.tile([64, 512], DT, tag="bcA_sb")
            nc.vector.tensor_copy(bcA_sb, bcA)
            bcB = ps2.tile([64, 512], fp32, tag="bc", bufs=1)
            nc.tensor.matmul(bcB, ones[0:1, :], recipB, start=True, stop=True)
            bcB_sb = rpool.tile([64, 512], DT, tag="bcB_sb")
            nc.vector.tensor_copy(bcB_sb, bcB)
            atA = rpool.tile([64, 512], DT, tag="atA")
            nc.vector.tensor_mul(atA, avA[0:64, :], bcA_sb)
            atB = rpool.tile([64, 512], DT, tag="atB")
            nc.vector.tensor_mul(atB, avB[0:64, :], bcB_sb)
            nc.sync.dma_start(out=apart[j, hp * 128:hp * 128 + 64, :], in_=atA)
            nc.sync.dma_start(out=apart[j, hp * 128 + 64:hp * 128 + 128, :], in_=atB)
        nc.gpsimd.collective_compute(
            "AllGather", mybir.AluOpType.bypass,
            replica_groups=PAIRS,
            ins=[apart[j].opt()], outs=[agath[j].opt()],
        )
        # gathered A^T back to SBUF: all 8 global head-pair chunks
        af = afull.tile([128, 8, QTW], DT, tag="af")
        nc.sync.dma_start(
            out=af, in_=agath[j].rearrange("(c p) q -> p c q", p=128))
        # out-proj for q-tile j: this rank's 512 output columns only
        for qs in range(4):
            q0 = qs * 128
            yp = ps2.tile([128, 512], fp32, tag="yp", bufs=1)
            for c in range(8):
                nc.tensor.matmul(yp, af[:, c, q0:q0 + 128],
                                 wo_sb[:, c, :], start=(c == 0), stop=False)
            nc.tensor.matmul(yp, ones_row, bo_dt, start=False, stop=True)
            ysb = ypool.tile([128, ESH], DT, tag="ysb")
            nc.vector.tensor_copy(ysb, yp)
            nc.sync.dma_start(out=io["y"].ap()[j * QTW + q0:j * QTW + q0 + 128, :],
                              in_=ysb)

    for cm in (dram_cm, ps2_cm, ypool_cm, afull_cm, rpool_cm, slabs_cm,
               kv_cm, const_cm):
        cm.__exit__(None, None, None)


def build():
    nc = bacc.Bacc("TRN2", target_bir_lowering=False, debug=False,
                   num_devices=8)
    io = {
        "xt": nc.dram_tensor("xt", [E, S], bf16, kind="ExternalInput"),
        "wq": nc.dram_tensor("wq", [E, ESH], bf16, kind="ExternalInput"),
        "wk": nc.dram_tensor("wk", [E, ESH], bf16, kind="ExternalInput"),
        "wv": nc.dram_tensor("wv", [E, ESH], bf16, kind="ExternalInput"),
        "wo": nc.dram_tensor("wo", [E, ESH], bf16, kind="ExternalInput"),
        "bo": nc.dram_tensor("bo", [ESH], bf16, kind="ExternalInput"),
        "masks": nc.dram_tensor("masks", [128, 896], bf16, kind="ExternalInput"),
        "y": nc.dram_tensor("y", [S, ESH], bf16, kind="ExternalOutput"),
    }
    with tile.TileContext(nc) as tc:
        _body(tc, io)
    nc.finalize()
    return nc


def make_in_maps(x, Wq, Wk, Wv, Wo, bo):
    """Shard full inputs into the 8 per-core input maps (bf16)."""
    x = np.asarray(x, dtype=np.float32)
    Wq, Wk, Wv, Wo = (np.asarray(w, dtype=np.float32) for w in (Wq, Wk, Wv, Wo))
    bo = np.asarray(bo, dtype=np.float32)
    kp = np.arange(128)[:, None]
    u = np.arange(896)[None, :]
    masks = (u >= kp + 384).astype(NPDT)
    in_maps = []
    for c in range(8):
        b, g = c // 2, c % 2
        csl = slice(g * ESH, (g + 1) * ESH)
        in_maps.append({
            "xt": np.ascontiguousarray(x[b].T).astype(NPDT),
            "wq": np.ascontiguousarray(Wq[:, csl]).astype(NPDT),
            "wk": np.ascontiguousarray(Wk[:, csl]).astype(NPDT),
            "wv": np.ascontiguousarray(Wv[:, csl]).astype(NPDT),
            "wo": np.ascontiguousarray(Wo[:, csl]).astype(NPDT),
            "bo": np.ascontiguousarray(bo[csl]).astype(NPDT),
            "masks": masks,
        })
    return in_maps


def kernel(x, Wq, Wk, Wv, Wo, bo):
    nc = build()
    in_maps = make_in_maps(x, Wq, Wk, Wv, Wo, bo)
    res = run_bass_kernel_spmd(nc, in_maps, core_ids=list(range(8)))
    y = np.empty((B, S, E), dtype=np.float32)
    for b in range(B):
        for g in range(2):
            y[b, :, g * ESH:(g + 1) * ESH] = res.results[2 * b + g]["y"].astype(
                np.float32)
    return y


# revision 3
# speedup vs baseline: 54.1881x; 15.8626x over previous
"""Causal multi-head attention kernel for Trainium2, 8 NeuronCores — v2.

Problem: x[4,2048,1024] fp32, Wq/Wk/Wv/Wo[1024,1024], bo[1024].
  y = softmax(causal(Q K^T)/sqrt(64)) V @ Wo + bo, H=16 heads of D=64.

Sharding: data-parallel over batch (4) x tensor-parallel over heads
(2 groups of 8). Core c handles batch c//2, heads (c%2)*8..+8.

v2 changes vs v1 (which AllReduced full y per q-tile chunk):
  - All I/O shipped as bf16 (halves per-execute host<->device traffic,
    which dominates the wall clock under the axon relay).
  - All matmuls run in bf16 (PE full rate, fp32 PSUM accumulation).
  - Out-proj: instead of computing full y partial sums + AllReduce
    (2N wire), each TP pair AllGathers the normalized attention
    outputs A^T per q-tile (N/2 wire) and each rank computes only its
    512 output columns: y[:, g*512:+512] = A_full @ Wo[:, g-half].
    The gathered A^T is read back uniformly on both ranks (same
    program), so out-proj consumes only the gathered copy.
  - Each core returns a [2048, 512] bf16 y column-shard (quarter the
    v1 output bytes); the host assembles and casts to fp32.

Per-core layout (unchanged core ideas from v1):
  - x^T shipped pre-transposed; projections contract over E directly.
  - Q^T/K^T stored [128, S] per local head-pair (rows 0-63 = head 2hp,
    64-127 = head 2hp+1); scores computed transposed as K Q^T with the
    two heads running as row-packed concurrent K=64 matmuls.
  - exp on ACT reads both PSUM banks [128,1024] in one instruction;
    causal masking multiplies the 4 diagonal-band blocks by slices of
    one precomputed step-pattern ("mega-mask").
  - AV accumulated transposed with a ones-column per V tile producing
    softmax denominators in row 64; normalization deferred to after AV
    via reciprocal + K=1 ones-matmul partition broadcast.
"""

import numpy as np
import ml_dtypes

import concourse.bass as bass
from concourse import bacc
import concourse.mybir as mybir
import concourse.tile as tile
from concourse.bass_utils import run_bass_kernel_spmd

B, S, E, H, D = 4, 2048, 1024, 16, 64
ESH = 512           # per-core E shard (8 heads x 64)
HP = 4              # head pairs per core
NJ, QTW = 4, 512    # q tiles
NKB, KBW = 16, 128  # k blocks

fp32 = mybir.dt.float32
bf16 = mybir.dt.bfloat16
DT = bf16
NPDT = ml_dtypes.bfloat16
AF = mybir.ActivationFunctionType
PAIRS = [[0, 1], [2, 3], [4, 5], [6, 7]]


def _body(tc, io):
    nc = tc.nc

    const_cm = tc.tile_pool(name="const", bufs=1)
    const = const_cm.__enter__()
    ones = const.tile([1, 64], DT, tag="ones")
    nc.vector.memset(ones, 1.0)
    ones_row = const.tile([1, 128], DT, tag="ones_row")
    nc.vector.memset(ones_row, 1.0)
    # mega-mask M[kp,u] = (u >= kp+384); diag pattern r slice = [384-128r:+512]
    masks = const.tile([128, 896], DT, tag="masks")
    nc.sync.dma_start(out=masks, in_=io["masks"].ap())
    bo_dt = const.tile([1, ESH], DT, tag="bo_dt")
    nc.sync.dma_start(out=bo_dt, in_=io["bo"].ap()[None, :])
    # Wo: [1024 rows = all 8 global head-pair chunks, 512 cols = g-half]
    wo_sb = const.tile([128, 8, ESH], DT, tag="wo")
    nc.sync.dma_start(out=wo_sb, in_=io["wo"].ap().rearrange("(c p) e -> p c e", p=128))

    kv_cm = tc.tile_pool(name="kv", bufs=1)
    kv = kv_cm.__enter__()
    qt_sb = [kv.tile([128, S], DT, tag=f"qt{hp}", name=f"qt{hp}") for hp in range(HP)]
    kt_sb = [kv.tile([128, S], DT, tag=f"kt{hp}", name=f"kt{hp}") for hp in range(HP)]
    v_sb = [kv.tile([128, 8, 65], DT, tag=f"v{kb}", name=f"v{kb}") for kb in range(NKB)]

    # ---------------- phase 1: projections ----------------
    wpool_cm = tc.tile_pool(name="wpool", bufs=1)
    wpool = wpool_cm.__enter__()
    wq_sb = wpool.tile([128, 8, ESH], DT, tag="wq")
    wk_sb = wpool.tile([128, 8, ESH], DT, tag="wk")
    wv_sb = wpool.tile([128, 8, ESH], DT, tag="wv")
    for t, nm in ((wq_sb, "wq"), (wk_sb, "wk"), (wv_sb, "wv")):
        nc.sync.dma_start(out=t, in_=io[nm].ap().rearrange("(e p) c -> p e c", p=128))

    xpool_cm = tc.tile_pool(name="xpool", bufs=2)
    xpool = xpool_cm.__enter__()
    ps1_cm = tc.tile_pool(name="ps1", bufs=1, space="PSUM")
    ps1 = ps1_cm.__enter__()

    for st_i in range(4):
        ssl = slice(st_i * 512, (st_i + 1) * 512)
        # x^T page for this s-tile: [128, 8 E-chunks, 512]
        xt = xpool.tile([128, 8, 512], DT, tag="xt")
        nc.sync.dma_start(
            out=xt,
            in_=io["xt"].ap().rearrange("(e p) s -> p e s", p=128)[:, :, ssl])
        for hp in range(HP):
            psq = ps1.tile([128, 512], fp32, tag="pj", bufs=3)
            for e in range(8):
                nc.tensor.matmul(psq, wq_sb[:, e, hp * 128:(hp + 1) * 128],
                                 xt[:, e, :], start=(e == 0), stop=(e == 7))
            nc.vector.tensor_copy(qt_sb[hp][:, ssl], psq)
            psk = ps1.tile([128, 512], fp32, tag="pj", bufs=3)
            for e in range(8):
                nc.tensor.matmul(psk, wk_sb[:, e, hp * 128:(hp + 1) * 128],
                                 xt[:, e, :], start=(e == 0), stop=(e == 7))
            nc.vector.tensor_copy(kt_sb[hp][:, ssl], psk)
        for sb in range(4):
            psv = ps1.tile([128, 512], fp32, tag="pj", bufs=3)
            for e in range(8):
                nc.tensor.matmul(psv, xt[:, e, sb * 128:(sb + 1) * 128],
                                 wv_sb[:, e, :], start=(e == 0), stop=(e == 7))
            kb = st_i * 4 + sb
            nc.vector.tensor_copy(
                v_sb[kb][:, :, 0:64],
                psv.rearrange("p (h d) -> p h d", h=8))
            nc.vector.memset(v_sb[kb][:, :, 64:65], 1.0)

    ps1_cm.__exit__(None, None, None)
    xpool_cm.__exit__(None, None, None)
    wpool_cm.__exit__(None, None, None)

    # -------- phase 2+3: attention, A^T AllGather, half out-proj --------
    slabs_cm = tc.tile_pool(name="slabs", bufs=3)
    slabs = slabs_cm.__enter__()
    rpool_cm = tc.tile_pool(name="rpool", bufs=2)
    rpool = rpool_cm.__enter__()
    afull_cm = tc.tile_pool(name="afull", bufs=2)
    afull = afull_cm.__enter__()
    ypool_cm = tc.tile_pool(name="ypool", bufs=3)
    ypool = ypool_cm.__enter__()
    ps2_cm = tc.tile_pool(name="ps2", bufs=1, space="PSUM")
    ps2 = ps2_cm.__enter__()
    dram_cm = tc.tile_pool(name="dram", bufs=1, space="DRAM")
    dram = dram_cm.__enter__()
    apart = dram.tile([NJ, ESH, QTW], DT)    # local A^T per q-tile
    agath = dram.tile([NJ, 2 * ESH, QTW], DT)  # pair-gathered A^T

    for j in range(NJ):
        jsl = slice(j * QTW, (j + 1) * QTW)
        for hp in range(HP):
            # two heads accumulate in separate base-0 PSUM tiles; row 64
            # collects softmax denominators via the V ones-column.
            avA = ps2.tile([65, 512], fp32, tag="avA", bufs=1)
            avB = ps2.tile([65, 512], fp32, tag="avB", bufs=1)
            kmax = 4 * j + 4
            for kb in range(kmax):
                ksl = slice(kb * KBW, (kb + 1) * KBW)
                sc = ps2.tile([128, 1024], fp32, tag="sc", bufs=2)
                nc.tensor.matmul(sc[:, 0:512], kt_sb[hp][0:64, ksl],
                                 qt_sb[hp][0:64, jsl], start=True, stop=True)
                nc.tensor.matmul(sc[:, 512:1024], kt_sb[hp][64:128, ksl],
                                 qt_sb[hp][64:128, jsl], start=True, stop=True)
                slab = slabs.tile([128, 1024], DT, tag="slab")
                nc.scalar.activation(slab, sc, AF.Exp, bias=0.0, scale=0.125)
                r = kb - 4 * j
                if r >= 0:
                    msl = slice(384 - 128 * r, 384 - 128 * r + 512)
                    nc.vector.tensor_mul(slab[:, 0:512], slab[:, 0:512],
                                         masks[:, msl])
                    nc.vector.tensor_mul(slab[:, 512:1024], slab[:, 512:1024],
                                         masks[:, msl])
                first, last = kb == 0, kb == kmax - 1
                nc.tensor.matmul(avA, v_sb[kb][:, 2 * hp, :],
                                 slab[:, 0:512], start=first, stop=last)
                nc.tensor.matmul(avB, v_sb[kb][:, 2 * hp + 1, :],
                                 slab[:, 512:1024], start=first, stop=last)
            recipA = rpool.tile([1, 512], DT, tag="recipA")
            recipB = rpool.tile([1, 512], DT, tag="recipB")
            with nc.allow_low_precision(reason="bf16 softmax denom recip"):
                nc.vector.reciprocal(recipA, avA[64:65, :])
                nc.vector.reciprocal(recipB, avB[64:65, :])
            bcA = ps2.tile([64, 512], fp32, tag="bc", bufs=1)
            nc.tensor.matmul(bcA, ones[0:1, :], recipA, start=True, stop=True)
            # DVE reads at most one PSUM operand: stage bc in SBUF
            bcA_sb = rpool.tile([64, 512], DT, tag="bcA_sb")
            nc.vector.tensor_copy(bcA_sb, bcA)
            bcB = ps2.tile([64, 512], fp32, tag="bc", bufs=1)
            nc.tensor.matmul(bcB, ones[0:1, :], recipB, start=True, stop=True)
            bcB_sb = rpool.tile([64, 512], DT, tag="bcB_sb")
            nc.vector.tensor_copy(bcB_sb, bcB)
            atA = rpool.tile([64, 512], DT, tag="atA")
            nc.vector.tensor_mul(atA, avA[0:64, :], bcA_sb)
            atB = rpool.tile([64, 512], DT, tag="atB")
            nc.vector.tensor_mul(atB, avB[0:64, :], bcB_sb)
            nc.sync.dma_start(out=apart[j, hp * 128:hp * 128 + 64, :], in_=atA)
            nc.sync.dma_start(out=apart[j, hp * 128 + 64:hp * 128 + 128, :], in_=atB)
        nc.gpsimd.collective_compute(
            "AllGather", mybir.AluOpType.bypass,
            replica_groups=PAIRS,
            ins=[apart[j].opt()], outs=[agath[j].opt()],
        )
        # gathered A^T back to SBUF: all 8 global head-pair chunks
        af = afull.tile([128, 8, QTW], DT, tag="af")
        nc.sync.dma_start(
            out=af, in_=agath[j].rearrange("(c p) q -> p c q", p=128))
        # out-proj for q-tile j: this rank's 512 output columns only
        for qs in range(4):
            q0 = qs * 128
            yp = ps2.tile([128, 512], fp32, tag="yp", bufs=1)
            for c in range(8):
                nc.tensor.matmul(yp, af[:, c, q0:q0 + 128],
                                 wo_sb[:, c, :], start=(c == 0), stop=False)
            nc.tensor.matmul(yp, ones_row, bo_dt, start=False, stop=True)
            ysb = ypool.tile([128, ESH], DT, tag="ysb")
            nc.vector.tensor_copy(ysb, yp)
            nc.sync.dma_start(out=io["y"].ap()[j * QTW + q0:j * QTW + q0 + 128, :],
                              in_=ysb)

    for cm in (dram_cm, ps2_cm, ypool_cm, afull_cm, rpool_cm, slabs_cm,
               kv_cm, const_cm):
        cm.__exit__(None, None, None)


def build(repeat=1):
    """repeat>1 emits the whole kernel body N times back-to-back in one
    NEFF - used only for timing (amortizes per-dispatch overhead so the
    steady-state per-body time approaches true HW execution time)."""
    nc = bacc.Bacc("TRN2", target_bir_lowering=False, debug=False,
                   num_devices=8)
    io = {
        "xt": nc.dram_tensor("xt", [E, S], bf16, kind="ExternalInput"),
        "wq": nc.dram_tensor("wq", [E, ESH], bf16, kind="ExternalInput"),
        "wk": nc.dram_tensor("wk", [E, ESH], bf16, kind="ExternalInput"),
        "wv": nc.dram_tensor("wv", [E, ESH], bf16, kind="ExternalInput"),
        "wo": nc.dram_tensor("wo", [E, ESH], bf16, kind="ExternalInput"),
        "bo": nc.dram_tensor("bo", [ESH], bf16, kind="ExternalInput"),
        "masks": nc.dram_tensor("masks", [128, 896], bf16, kind="ExternalInput"),
        "y": nc.dram_tensor("y", [S, ESH], bf16, kind="ExternalOutput"),
    }
    with tile.TileContext(nc) as tc:
        for _ in range(repeat):
            _body(tc, io)
    nc.finalize()
    return nc


def make_in_maps(x, Wq, Wk, Wv, Wo, bo):
    """Shard full inputs into the 8 per-core input maps (bf16)."""
    x = np.asarray(x, dtype=np.float32)
    Wq, Wk, Wv, Wo = (np.asarray(w, dtype=np.float32) for w in (Wq, Wk, Wv, Wo))
    bo = np.asarray(bo, dtype=np.float32)
    kp = np.arange(128)[:, None]
    u = np.arange(896)[None, :]
    masks = (u >= kp + 384).astype(NPDT)
    in_maps = []
    for c in range(8):
        b, g = c // 2, c % 2
        csl = slice(g * ESH, (g + 1) * ESH)
        in_maps.append({
            "xt": np.ascontiguousarray(x[b].T).astype(NPDT),
            "wq": np.ascontiguousarray(Wq[:, csl]).astype(NPDT),
            "wk": np.ascontiguousarray(Wk[:, csl]).astype(NPDT),
            "wv": np.ascontiguousarray(Wv[:, csl]).astype(NPDT),
            "wo": np.ascontiguousarray(Wo[:, csl]).astype(NPDT),
            "bo": np.ascontiguousarray(bo[csl]).astype(NPDT),
            "masks": masks,
        })
    return in_maps


def kernel(x, Wq, Wk, Wv, Wo, bo):
    nc = build()
    in_maps = make_in_maps(x, Wq, Wk, Wv, Wo, bo)
    res = run_bass_kernel_spmd(nc, in_maps, core_ids=list(range(8)))
    y = np.empty((B, S, E), dtype=np.float32)
    for b in range(B):
        for g in range(2):
            y[b, :, g * ESH:(g + 1) * ESH] = res.results[2 * b + g]["y"].astype(
                np.float32)
    return y


# revision 4
# speedup vs baseline: 59.8131x; 1.1038x over previous
"""Causal multi-head attention kernel for Trainium2, 8 NeuronCores — v2.

Problem: x[4,2048,1024] fp32, Wq/Wk/Wv/Wo[1024,1024], bo[1024].
  y = softmax(causal(Q K^T)/sqrt(64)) V @ Wo + bo, H=16 heads of D=64.

Sharding: data-parallel over batch (4) x tensor-parallel over heads
(2 groups of 8). Core c handles batch c//2, heads (c%2)*8..+8.

v2 changes vs v1 (which AllReduced full y per q-tile chunk):
  - All I/O shipped as bf16 (halves per-execute host<->device traffic,
    which dominates the wall clock under the axon relay).
  - All matmuls run in bf16 (PE full rate, fp32 PSUM accumulation).
  - Out-proj: instead of computing full y partial sums + AllReduce
    (2N wire), each TP pair AllGathers the normalized attention
    outputs A^T per q-tile (N/2 wire) and each rank computes only its
    512 output columns: y[:, g*512:+512] = A_full @ Wo[:, g-half].
    The gathered A^T is read back uniformly on both ranks (same
    program), so out-proj consumes only the gathered copy.
  - Each core returns a [2048, 512] bf16 y column-shard (quarter the
    v1 output bytes); the host assembles and casts to fp32.

Per-core layout (unchanged core ideas from v1):
  - x^T shipped pre-transposed; projections contract over E directly.
  - Q^T/K^T stored [128, S] per local head-pair (rows 0-63 = head 2hp,
    64-127 = head 2hp+1); scores computed transposed as K Q^T with the
    two heads running as row-packed concurrent K=64 matmuls.
  - exp on ACT reads both PSUM banks [128,1024] in one instruction;
    causal masking multiplies the 4 diagonal-band blocks by slices of
    one precomputed step-pattern ("mega-mask").
  - AV accumulated transposed with a ones-column per V tile producing
    softmax denominators in row 64; normalization deferred to after AV
    via reciprocal + K=1 ones-matmul partition broadcast.
"""

import numpy as np
import ml_dtypes

import concourse.bass as bass
from concourse import bacc
import concourse.mybir as mybir
import concourse.tile as tile
from concourse.bass_utils import run_bass_kernel_spmd

B, S, E, H, D = 4, 2048, 1024, 16, 64
ESH = 512           # per-core E shard (8 heads x 64)
HP = 4              # head pairs per core
NJ, QTW = 4, 512    # q tiles
NKB, KBW = 16, 128  # k blocks

fp32 = mybir.dt.float32
bf16 = mybir.dt.bfloat16
DT = bf16
NPDT = ml_dtypes.bfloat16
AF = mybir.ActivationFunctionType
PAIRS = [[0, 1], [2, 3], [4, 5], [6, 7]]


def _body(tc, io):
    nc = tc.nc

    const_cm = tc.tile_pool(name="const", bufs=1)
    const = const_cm.__enter__()
    ones = const.tile([1, 64], DT, tag="ones")
    nc.vector.memset(ones, 1.0)
    ones_row = const.tile([1, 128], DT, tag="ones_row")
    nc.vector.memset(ones_row, 1.0)
    # mega-mask M[kp,u] = (u >= kp+384); diag pattern r slice = [384-128r:+512]
    masks = const.tile([128, 896], DT, tag="masks")
    nc.sync.dma_start(out=masks, in_=io["masks"].ap())
    bo_dt = const.tile([1, ESH], DT, tag="bo_dt")
    nc.sync.dma_start(out=bo_dt, in_=io["bo"].ap()[None, :])
    # Wo: [1024 rows = all 8 global head-pair chunks, 512 cols = g-half]
    wo_sb = const.tile([128, 8, ESH], DT, tag="wo")
    nc.sync.dma_start(out=wo_sb, in_=io["wo"].ap().rearrange("(c p) e -> p c e", p=128))

    kv_cm = tc.tile_pool(name="kv", bufs=1)
    kv = kv_cm.__enter__()
    qt_sb = [kv.tile([128, S], DT, tag=f"qt{hp}", name=f"qt{hp}") for hp in range(HP)]
    kt_sb = [kv.tile([128, S], DT, tag=f"kt{hp}", name=f"kt{hp}") for hp in range(HP)]
    v_sb = [kv.tile([128, 8, 65], DT, tag=f"v{kb}", name=f"v{kb}") for kb in range(NKB)]

    # ---------------- phase 1: projections ----------------
    wpool_cm = tc.tile_pool(name="wpool", bufs=1)
    wpool = wpool_cm.__enter__()
    wq_sb = wpool.tile([128, 8, ESH], DT, tag="wq")
    wk_sb = wpool.tile([128, 8, ESH], DT, tag="wk")
    wv_sb = wpool.tile([128, 8, ESH], DT, tag="wv")
    for t, nm in ((wq_sb, "wq"), (wk_sb, "wk"), (wv_sb, "wv")):
        nc.sync.dma_start(out=t, in_=io[nm].ap().rearrange("(e p) c -> p e c", p=128))

    xpool_cm = tc.tile_pool(name="xpool", bufs=2)
    xpool = xpool_cm.__enter__()
    ps1_cm = tc.tile_pool(name="ps1", bufs=1, space="PSUM")
    ps1 = ps1_cm.__enter__()

    for st_i in range(4):
        ssl = slice(st_i * 512, (st_i + 1) * 512)
        # x^T page for this s-tile: [128, 8 E-chunks, 512]
        xt = xpool.tile([128, 8, 512], DT, tag="xt")
        nc.sync.dma_start(
            out=xt,
            in_=io["xt"].ap().rearrange("(e p) s -> p e s", p=128)[:, :, ssl])
        for hp in range(HP):
            psq = ps1.tile([128, 512], fp32, tag="pj", bufs=3)
            for e in range(8):
                nc.tensor.matmul(psq, wq_sb[:, e, hp * 128:(hp + 1) * 128],
                                 xt[:, e, :], start=(e == 0), stop=(e == 7))
            nc.vector.tensor_copy(qt_sb[hp][:, ssl], psq)
            psk = ps1.tile([128, 512], fp32, tag="pj", bufs=3)
            for e in range(8):
                nc.tensor.matmul(psk, wk_sb[:, e, hp * 128:(hp + 1) * 128],
                                 xt[:, e, :], start=(e == 0), stop=(e == 7))
            nc.vector.tensor_copy(kt_sb[hp][:, ssl], psk)
        for sb in range(4):
            psv = ps1.tile([128, 512], fp32, tag="pj", bufs=3)
            for e in range(8):
                nc.tensor.matmul(psv, xt[:, e, sb * 128:(sb + 1) * 128],
                                 wv_sb[:, e, :], start=(e == 0), stop=(e == 7))
            kb = st_i * 4 + sb
            nc.vector.tensor_copy(
                v_sb[kb][:, :, 0:64],
                psv.rearrange("p (h d) -> p h d", h=8))
            nc.vector.memset(v_sb[kb][:, :, 64:65], 1.0)

    ps1_cm.__exit__(None, None, None)
    xpool_cm.__exit__(None, None, None)
    wpool_cm.__exit__(None, None, None)

    # -------- phase 2+3: attention, A^T AllGather, half out-proj --------
    slabs_cm = tc.tile_pool(name="slabs", bufs=3)
    slabs = slabs_cm.__enter__()
    rpool_cm = tc.tile_pool(name="rpool", bufs=2)
    rpool = rpool_cm.__enter__()
    afull_cm = tc.tile_pool(name="afull", bufs=2)
    afull = afull_cm.__enter__()
    ypool_cm = tc.tile_pool(name="ypool", bufs=3)
    ypool = ypool_cm.__enter__()
    ps2_cm = tc.tile_pool(name="ps2", bufs=1, space="PSUM")
    ps2 = ps2_cm.__enter__()
    dram_cm = tc.tile_pool(name="dram", bufs=1, space="DRAM")
    dram = dram_cm.__enter__()
    apart = dram.tile([NJ, ESH, QTW], DT)    # local A^T per q-tile
    agath = dram.tile([NJ, 2 * ESH, QTW], DT)  # pair-gathered A^T

    for j in range(NJ):
        jsl = slice(j * QTW, (j + 1) * QTW)
        for hp in range(HP):
            # two heads accumulate in separate base-0 PSUM tiles; row 64
            # collects softmax denominators via the V ones-column.
            avA = ps2.tile([65, 512], fp32, tag="avA", bufs=1)
            avB = ps2.tile([65, 512], fp32, tag="avB", bufs=1)
            kmax = 4 * j + 4
            for kb in range(kmax):
                ksl = slice(kb * KBW, (kb + 1) * KBW)
                sc = ps2.tile([128, 1024], fp32, tag="sc", bufs=2)
                nc.tensor.matmul(sc[:, 0:512], kt_sb[hp][0:64, ksl],
                                 qt_sb[hp][0:64, jsl], start=True, stop=True)
                nc.tensor.matmul(sc[:, 512:1024], kt_sb[hp][64:128, ksl],
                                 qt_sb[hp][64:128, jsl], start=True, stop=True)
                slab = slabs.tile([128, 1024], DT, tag="slab")
                nc.scalar.activation(slab, sc, AF.Exp, bias=0.0, scale=0.125)
                r = kb - 4 * j
                if r >= 0:
                    msl = slice(384 - 128 * r, 384 - 128 * r + 512)
                    nc.vector.tensor_mul(slab[:, 0:512], slab[:, 0:512],
                                         masks[:, msl])
                    nc.vector.tensor_mul(slab[:, 512:1024], slab[:, 512:1024],
                                         masks[:, msl])
                first, last = kb == 0, kb == kmax - 1
                nc.tensor.matmul(avA, v_sb[kb][:, 2 * hp, :],
                                 slab[:, 0:512], start=first, stop=last)
                nc.tensor.matmul(avB, v_sb[kb][:, 2 * hp + 1, :],
                                 slab[:, 512:1024], start=first, stop=last)
            recipA = rpool.tile([1, 512], DT, tag="recipA")
            recipB = rpool.tile([1, 512], DT, tag="recipB")
            with nc.allow_low_precision(reason="bf16 softmax denom recip"):
                nc.vector.reciprocal(recipA, avA[64:65, :])
                nc.vector.reciprocal(recipB, avB[64:65, :])
            bcA = ps2.tile([64, 512], fp32, tag="bc", bufs=1)
            nc.tensor.matmul(bcA, ones[0:1, :], recipA, start=True, stop=True)
            # DVE reads at most one PSUM operand: stage bc in SBUF
            bcA_sb = rpool.tile([64, 512], DT, tag="bcA_sb")
            nc.vector.tensor_copy(bcA_sb, bcA)
            bcB = ps2.tile([64, 512], fp32, tag="bc", bufs=1)
            nc.tensor.matmul(bcB, ones[0:1, :], recipB, start=True, stop=True)
            bcB_sb = rpool.tile([64, 512], DT, tag="bcB_sb")
            nc.vector.tensor_copy(bcB_sb, bcB)
            atA = rpool.tile([64, 512], DT, tag="atA")
            nc.vector.tensor_mul(atA, avA[0:64, :], bcA_sb)
            atB = rpool.tile([64, 512], DT, tag="atB")
            nc.vector.tensor_mul(atB, avB[0:64, :], bcB_sb)
            nc.sync.dma_start(out=apart[j, hp * 128:hp * 128 + 64, :], in_=atA)
            nc.sync.dma_start(out=apart[j, hp * 128 + 64:hp * 128 + 128, :], in_=atB)
        nc.gpsimd.collective_compute(
            "AllGather", mybir.AluOpType.bypass,
            replica_groups=PAIRS,
            ins=[apart[j].opt()], outs=[agath[j].opt()],
        )
        # gathered A^T back to SBUF: all 8 global head-pair chunks
        af = afull.tile([128, 8, QTW], DT, tag="af")
        nc.sync.dma_start(
            out=af, in_=agath[j].rearrange("(c p) q -> p c q", p=128))
        # out-proj for q-tile j: this rank's 512 output columns only
        for qs in range(4):
            q0 = qs * 128
            yp = ps2.tile([128, 512], fp32, tag="yp", bufs=1)
            for c in range(8):
                nc.tensor.matmul(yp, af[:, c, q0:q0 + 128],
                                 wo_sb[:, c, :], start=(c == 0), stop=False)
            nc.tensor.matmul(yp, ones_row, bo_dt, start=False, stop=True)
            ysb = ypool.tile([128, ESH], DT, tag="ysb")
            nc.vector.tensor_copy(ysb, yp)
            nc.sync.dma_start(out=io["y"].ap()[j * QTW + q0:j * QTW + q0 + 128, :],
                              in_=ysb)

    for cm in (dram_cm, ps2_cm, ypool_cm, afull_cm, rpool_cm, slabs_cm,
               kv_cm, const_cm):
        cm.__exit__(None, None, None)


def build(repeat=1):
    """repeat>1 emits the whole kernel body N times back-to-back in one
    NEFF - used only for timing (amortizes per-dispatch overhead so the
    steady-state per-body time approaches true HW execution time)."""
    nc = bacc.Bacc("TRN2", target_bir_lowering=False, debug=False,
                   num_devices=8)
    io = {
        "xt": nc.dram_tensor("xt", [E, S], bf16, kind="ExternalInput"),
        "wq": nc.dram_tensor("wq", [E, ESH], bf16, kind="ExternalInput"),
        "wk": nc.dram_tensor("wk", [E, ESH], bf16, kind="ExternalInput"),
        "wv": nc.dram_tensor("wv", [E, ESH], bf16, kind="ExternalInput"),
        "wo": nc.dram_tensor("wo", [E, ESH], bf16, kind="ExternalInput"),
        "bo": nc.dram_tensor("bo", [ESH], bf16, kind="ExternalInput"),
        "masks": nc.dram_tensor("masks", [128, 896], bf16, kind="ExternalInput"),
        "y": nc.dram_tensor("y", [S, ESH], bf16, kind="ExternalOutput"),
    }
    with tile.TileContext(nc) as tc:
        for _ in range(repeat):
            _body(tc, io)
    nc.finalize()
    return nc


def make_in_maps(x, Wq, Wk, Wv, Wo, bo):
    """Shard full inputs into the 8 per-core input maps (bf16)."""
    x = np.asarray(x, dtype=np.float32)
    Wq, Wk, Wv, Wo = (np.asarray(w, dtype=np.float32) for w in (Wq, Wk, Wv, Wo))
    bo = np.asarray(bo, dtype=np.float32)
    kp = np.arange(128)[:, None]
    u = np.arange(896)[None, :]
    masks = (u >= kp + 384).astype(NPDT)
    in_maps = []
    for c in range(8):
        b, g = c // 2, c % 2
        csl = slice(g * ESH, (g + 1) * ESH)
        in_maps.append({
            "xt": np.ascontiguousarray(x[b].T).astype(NPDT),
            "wq": np.ascontiguousarray(Wq[:, csl]).astype(NPDT),
            "wk": np.ascontiguousarray(Wk[:, csl]).astype(NPDT),
            "wv": np.ascontiguousarray(Wv[:, csl]).astype(NPDT),
            "wo": np.ascontiguousarray(Wo[:, csl]).astype(NPDT),
            "bo": np.ascontiguousarray(bo[csl]).astype(NPDT),
            "masks": masks,
        })
    return in_maps


def kernel(x, Wq, Wk, Wv, Wo, bo):
    nc = build()
    in_maps = make_in_maps(x, Wq, Wk, Wv, Wo, bo)
    res = None
    for attempt in range(3):
        try:
            res = run_bass_kernel_spmd(nc, in_maps, core_ids=list(range(8)))
            break
        except Exception:
            # transient axon relay failures (mesh desync / worker hang-up)
            # recover on retry; re-raise only if persistent
            if attempt == 2:
                raise
            import time
            time.sleep(5)
    y = np.empty((B, S, E), dtype=np.float32)
    for b in range(B):
        for g in range(2):
            y[b, :, g * ESH:(g + 1) * ESH] = res.results[2 * b + g]["y"].astype(
                np.float32)
    return y


# revision 11
# speedup vs baseline: 132.3380x; 2.2125x over previous
"""Causal multi-head attention kernel for Trainium2, 8 NeuronCores — v2.

Problem: x[4,2048,1024] fp32, Wq/Wk/Wv/Wo[1024,1024], bo[1024].
  y = softmax(causal(Q K^T)/sqrt(64)) V @ Wo + bo, H=16 heads of D=64.

Sharding: data-parallel over batch (4) x tensor-parallel over heads
(2 groups of 8). Core c handles batch c//2, heads (c%2)*8..+8.

v2 changes vs v1 (which AllReduced full y per q-tile chunk):
  - All I/O shipped as bf16 (halves per-execute host<->device traffic,
    which dominates the wall clock under the axon relay).
  - All matmuls run in bf16 (PE full rate, fp32 PSUM accumulation).
  - Out-proj: instead of computing full y partial sums + AllReduce
    (2N wire), each TP pair AllGathers the normalized attention
    outputs A^T per q-tile (N/2 wire) and each rank computes only its
    512 output columns: y[:, g*512:+512] = A_full @ Wo[:, g-half].
    The gathered A^T is read back uniformly on both ranks (same
    program), so out-proj consumes only the gathered copy.
  - Each core returns a [2048, 512] bf16 y column-shard (quarter the
    v1 output bytes); the host assembles and casts to fp32.

Per-core layout (unchanged core ideas from v1):
  - x^T shipped pre-transposed; projections contract over E directly.
  - Q^T/K^T stored [128, S] per local head-pair (rows 0-63 = head 2hp,
    64-127 = head 2hp+1); scores computed transposed as K Q^T with the
    two heads running as row-packed concurrent K=64 matmuls.
  - exp on ACT reads both PSUM banks [128,1024] in one instruction;
    causal masking multiplies the 4 diagonal-band blocks by slices of
    one precomputed step-pattern ("mega-mask").
  - AV accumulated transposed with a ones-column per V tile producing
    softmax denominators in row 64; normalization deferred to after AV
    via reciprocal + K=1 ones-matmul partition broadcast.
"""

import numpy as np
import ml_dtypes

import concourse.bass as bass
from concourse import bacc
import concourse.mybir as mybir
import concourse.tile as tile
from concourse.bass_utils import run_bass_kernel_spmd

B, S, E, H, D = 4, 2048, 1024, 16, 64
ESH = 512           # per-core E shard (8 heads x 64)
HP = 4              # head pairs per core
NJ, QTW = 4, 512    # q tiles
NKB, KBW = 16, 128  # k blocks

fp32 = mybir.dt.float32
bf16 = mybir.dt.bfloat16
DT = bf16
NPDT = ml_dtypes.bfloat16
AF = mybir.ActivationFunctionType
PAIRS = [[0, 1], [2, 3], [4, 5], [6, 7]]


def _body(tc, io):
    nc = tc.nc

    const_cm = tc.tile_pool(name="const", bufs=1)
    const = const_cm.__enter__()
    ones = const.tile([1, 64], DT, tag="ones")
    nc.vector.memset(ones, 1.0)
    ones_row = const.tile([1, 128], DT, tag="ones_row")
    nc.vector.memset(ones_row, 1.0)
    # mega-mask M[kp,u] = (u >= kp+384); diag pattern r slice = [384-128r:+512]
    masks = const.tile([128, 896], DT, tag="masks")
    nc.sync.dma_start(out=masks, in_=io["masks"].ap())
    bo_dt = const.tile([1, ESH], DT, tag="bo_dt")
    nc.sync.dma_start(out=bo_dt, in_=io["bo"].ap()[None, :])
    wo_sb = const.tile([128, 8, ESH], DT, tag="wo")

    kv_cm = tc.tile_pool(name="kv", bufs=1)
    kv = kv_cm.__enter__()
    qt_sb = [kv.tile([128, S], DT, tag=f"qt{hp}", name=f"qt{hp}") for hp in range(HP)]
    kt_sb = [kv.tile([128, S], DT, tag=f"kt{hp}", name=f"kt{hp}") for hp in range(HP)]
    v_sb = [kv.tile([128, 8, 65], DT, tag=f"v{kb}", name=f"v{kb}") for kb in range(NKB)]

    # ---------------- phase 1: projections ----------------
    wpool_cm = tc.tile_pool(name="wpool", bufs=1)
    wpool = wpool_cm.__enter__()
    wq_sb = wpool.tile([128, 8, ESH], DT, tag="wq")
    wk_sb = wpool.tile([128, 8, ESH], DT, tag="wk")
    wv_sb = wpool.tile([128, 8, ESH], DT, tag="wv")
    for t, nm in ((wq_sb, "wq"), (wk_sb, "wk"), (wv_sb, "wv")):
        nc.sync.dma_start(out=t, in_=io[nm].ap().rearrange("(e p) c -> p e c", p=128))

    xpool_cm = tc.tile_pool(name="xpool", bufs=2)
    xpool = xpool_cm.__enter__()
    ps1_cm = tc.tile_pool(name="ps1", bufs=1, space="PSUM")
    ps1 = ps1_cm.__enter__()

    # Wo isn't needed until out-proj: load it after the phase-1-critical
    # wq/wk/wv DMAs so the first projection matmuls start sooner.
    # (emission here, tile lives in const pool above)
    nc.sync.dma_start(out=wo_sb, in_=io["wo"].ap().rearrange("(c p) e -> p c e", p=128))

    for st_i in range(4):
        ssl = slice(st_i * 512, (st_i + 1) * 512)
        # x^T page for this s-tile: [128, 8 E-chunks, 512]
        xt = xpool.tile([128, 8, 512], DT, tag="xt")
        nc.sync.dma_start(
            out=xt,
            in_=io["xt"].ap().rearrange("(e p) s -> p e s", p=128)[:, :, ssl])
        for hp in range(HP):
            psq = ps1.tile([128, 512], fp32, tag="pj", bufs=3)
            for e in range(8):
                nc.tensor.matmul(psq, wq_sb[:, e, hp * 128:(hp + 1) * 128],
                                 xt[:, e, :], start=(e == 0), stop=(e == 7))
            nc.vector.tensor_copy(qt_sb[hp][:, ssl], psq)
            psk = ps1.tile([128, 512], fp32, tag="pj", bufs=3)
            for e in range(8):
                nc.tensor.matmul(psk, wk_sb[:, e, hp * 128:(hp + 1) * 128],
                                 xt[:, e, :], start=(e == 0), stop=(e == 7))
            nc.vector.tensor_copy(kt_sb[hp][:, ssl], psk)
        for sb in range(4):
            psv = ps1.tile([128, 512], fp32, tag="pj", bufs=3)
            for e in range(8):
                nc.tensor.matmul(psv, xt[:, e, sb * 128:(sb + 1) * 128],
                                 wv_sb[:, e, :], start=(e == 0), stop=(e == 7))
            kb = st_i * 4 + sb
            nc.vector.tensor_copy(
                v_sb[kb][:, :, 0:64],
                psv.rearrange("p (h d) -> p h d", h=8))
            nc.vector.memset(v_sb[kb][:, :, 64:65], 1.0)

    ps1_cm.__exit__(None, None, None)
    xpool_cm.__exit__(None, None, None)
    wpool_cm.__exit__(None, None, None)

    # -------- phase 2+3: attention, A^T AllGather, half out-proj --------
    slabs_cm = tc.tile_pool(name="slabs", bufs=3)
    slabs = slabs_cm.__enter__()
    rpool_cm = tc.tile_pool(name="rpool", bufs=2)
    rpool = rpool_cm.__enter__()
    afull_cm = tc.tile_pool(name="afull", bufs=3)
    afull = afull_cm.__enter__()
    ypool_cm = tc.tile_pool(name="ypool", bufs=3)
    ypool = ypool_cm.__enter__()
    ps2_cm = tc.tile_pool(name="ps2", bufs=1, space="PSUM")
    ps2 = ps2_cm.__enter__()
    dram_cm = tc.tile_pool(name="dram", bufs=1, space="DRAM")
    dram = dram_cm.__enter__()
    apart = dram.tile([NJ, ESH, QTW], DT)    # local A^T per q-tile
    agath = dram.tile([NJ, 2 * ESH, QTW], DT)  # pair-gathered A^T

    def outproj(j, af):
        # out-proj for q-tile j: this rank's 512 output columns only
        for qs in range(4):
            q0 = qs * 128
            yp = ps2.tile([128, 512], fp32, tag="yp", bufs=1)
            for c in range(8):
                nc.tensor.matmul(yp, af[:, c, q0:q0 + 128],
                                 wo_sb[:, c, :], start=(c == 0), stop=False)
            nc.tensor.matmul(yp, ones_row, bo_dt, start=False, stop=True)
            ysb = ypool.tile([128, ESH], DT, tag="ysb")
            nc.vector.tensor_copy(ysb, yp)
            nc.sync.dma_start(out=io["y"].ap()[j * QTW + q0:j * QTW + q0 + 128, :],
                              in_=ysb)

    pending = []   # (j, af) whose out-proj is deferred two tiles so each
    # 40us AllGather overlaps two tiles of attention on PE
    for j in range(NJ):
        jsl = slice(j * QTW, (j + 1) * QTW)
        for hp in range(HP):
            # two heads accumulate in separate base-0 PSUM tiles; row 64
            # collects softmax denominators via the V ones-column.
            avA = ps2.tile([65, 512], fp32, tag="avA", bufs=1)
            avB = ps2.tile([65, 512], fp32, tag="avB", bufs=1)
            kmax = 4 * j + 4
            for kb in range(kmax):
                ksl = slice(kb * KBW, (kb + 1) * KBW)
                sc = ps2.tile([128, 1024], fp32, tag="sc", bufs=2)
                nc.tensor.matmul(sc[:, 0:512], kt_sb[hp][0:64, ksl],
                                 qt_sb[hp][0:64, jsl], start=True, stop=True)
                nc.tensor.matmul(sc[:, 512:1024], kt_sb[hp][64:128, ksl],
                                 qt_sb[hp][64:128, jsl], start=True, stop=True)
                slab = slabs.tile([128, 1024], DT, tag="slab")
                nc.scalar.activation(slab, sc, AF.Exp, bias=0.0, scale=0.125)
                r = kb - 4 * j
                if r >= 0:
                    msl = slice(384 - 128 * r, 384 - 128 * r + 512)
                    nc.vector.tensor_mul(slab[:, 0:512], slab[:, 0:512],
                                         masks[:, msl])
                    nc.vector.tensor_mul(slab[:, 512:1024], slab[:, 512:1024],
                                         masks[:, msl])
                first, last = kb == 0, kb == kmax - 1
                nc.tensor.matmul(avA, v_sb[kb][:, 2 * hp, :],
                                 slab[:, 0:512], start=first, stop=last)
                nc.tensor.matmul(avB, v_sb[kb][:, 2 * hp + 1, :],
                                 slab[:, 512:1024], start=first, stop=last)
            recipA = rpool.tile([1, 512], DT, tag="recipA")
            recipB = rpool.tile([1, 512], DT, tag="recipB")
            with nc.allow_low_precision(reason="bf16 softmax denom recip"):
                nc.vector.reciprocal(recipA, avA[64:65, :])
                nc.vector.reciprocal(recipB, avB[64:65, :])
            bcA = ps2.tile([64, 512], fp32, tag="bc", bufs=1)
            nc.tensor.matmul(bcA, ones[0:1, :], recipA, start=True, stop=True)
            # DVE reads at most one PSUM operand: stage bc in SBUF
            bcA_sb = rpool.tile([64, 512], DT, tag="bcA_sb")
            nc.vector.tensor_copy(bcA_sb, bcA)
            bcB = ps2.tile([64, 512], fp32, tag="bc", bufs=1)
            nc.tensor.matmul(bcB, ones[0:1, :], recipB, start=True, stop=True)
            bcB_sb = rpool.tile([64, 512], DT, tag="bcB_sb")
            nc.vector.tensor_copy(bcB_sb, bcB)
            atA = rpool.tile([64, 512], DT, tag="atA")
            nc.vector.tensor_mul(atA, avA[0:64, :], bcA_sb)
            atB = rpool.tile([64, 512], DT, tag="atB")
            nc.vector.tensor_mul(atB, avB[0:64, :], bcB_sb)
            nc.sync.dma_start(out=apart[j, hp * 128:hp * 128 + 64, :], in_=atA)
            nc.sync.dma_start(out=apart[j, hp * 128 + 64:hp * 128 + 128, :], in_=atB)
        nc.gpsimd.collective_compute(
            "AllGather", mybir.AluOpType.bypass,
            replica_groups=PAIRS,
            ins=[apart[j].opt()], outs=[agath[j].opt()],
        )
        # gathered A^T back to SBUF: all 8 global head-pair chunks
        af = afull.tile([128, 8, QTW], DT, tag="af")
        nc.sync.dma_start(
            out=af, in_=agath[j].rearrange("(c p) q -> p c q", p=128))
        pending.append((j, af))
        if len(pending) > 2:
            outproj(*pending.pop(0))
    for p in pending:
        outproj(*p)

    for cm in (dram_cm, ps2_cm, ypool_cm, afull_cm, rpool_cm, slabs_cm,
               kv_cm, const_cm):
        cm.__exit__(None, None, None)


def build(repeat=1):
    """repeat>1 emits the whole kernel body N times back-to-back in one
    NEFF - used only for timing (amortizes per-dispatch overhead so the
    steady-state per-body time approaches true HW execution time)."""
    nc = bacc.Bacc("TRN2", target_bir_lowering=False, debug=False,
                   num_devices=8)
    io = {
        "xt": nc.dram_tensor("xt", [E, S], bf16, kind="ExternalInput"),
        "wq": nc.dram_tensor("wq", [E, ESH], bf16, kind="ExternalInput"),
        "wk": nc.dram_tensor("wk", [E, ESH], bf16, kind="ExternalInput"),
        "wv": nc.dram_tensor("wv", [E, ESH], bf16, kind="ExternalInput"),
        "wo": nc.dram_tensor("wo", [E, ESH], bf16, kind="ExternalInput"),
        "bo": nc.dram_tensor("bo", [ESH], bf16, kind="ExternalInput"),
        "masks": nc.dram_tensor("masks", [128, 896], bf16, kind="ExternalInput"),
        "y": nc.dram_tensor("y", [S, ESH], bf16, kind="ExternalOutput"),
    }
    with tile.TileContext(nc) as tc:
        for _ in range(repeat):
            _body(tc, io)
    nc.finalize()
    return nc


def make_in_maps(x, Wq, Wk, Wv, Wo, bo):
    """Shard full inputs into the 8 per-core input maps (bf16)."""
    x = np.asarray(x, dtype=np.float32)
    Wq, Wk, Wv, Wo = (np.asarray(w, dtype=np.float32) for w in (Wq, Wk, Wv, Wo))
    bo = np.asarray(bo, dtype=np.float32)
    kp = np.arange(128)[:, None]
    u = np.arange(896)[None, :]
    masks = (u >= kp + 384).astype(NPDT)
    in_maps = []
    for c in range(8):
        b, g = c // 2, c % 2
        csl = slice(g * ESH, (g + 1) * ESH)
        in_maps.append({
            "xt": np.ascontiguousarray(x[b].T).astype(NPDT),
            "wq": np.ascontiguousarray(Wq[:, csl]).astype(NPDT),
            "wk": np.ascontiguousarray(Wk[:, csl]).astype(NPDT),
            "wv": np.ascontiguousarray(Wv[:, csl]).astype(NPDT),
            "wo": np.ascontiguousarray(Wo[:, csl]).astype(NPDT),
            "bo": np.ascontiguousarray(bo[csl]).astype(NPDT),
            "masks": masks,
        })
    return in_maps


def kernel(x, Wq, Wk, Wv, Wo, bo):
    nc = build()
    in_maps = make_in_maps(x, Wq, Wk, Wv, Wo, bo)
    res = None
    for attempt in range(3):
        try:
            res = run_bass_kernel_spmd(nc, in_maps, core_ids=list(range(8)))
            break
        except Exception:
            # transient axon relay failures (mesh desync / worker hang-up)
            # recover on retry; re-raise only if persistent
            if attempt == 2:
                raise
            import time
            time.sleep(5)
    y = np.empty((B, S, E), dtype=np.float32)
    for b in range(B):
        for g in range(2):
            y[b, :, g * ESH:(g + 1) * ESH] = res.results[2 * b + g]["y"].astype(
                np.float32)
    return y
